# revision 11
# baseline (speedup 1.0000x reference)
"""Trainium2 Bass kernel v2 for nn_LSTMMachine: tiny LSTM (H=2), B=8192, T=64, I=125.

Changes vs baseline v1:
  - image + W_ih shipped as fp16: halves HBM traffic (33MB -> 16.5MB/core)
    and enables Fast Weight Load on the PE stationaries (2x LDWEIGHTS).
  - ~2MB DMA chunks (better DMA efficiency).
  - (pe_accum=True variant exists but is numerically broken on HW —
    LDWEIGHTS inside an open PSUM accumulation group; default is off.)
"""

import sys

sys.path.insert(0, "/opt/trn_rl_repo")

import numpy as np

B, T, I, H = 8192, 64, 125, 2
NCORES = 8
P = 128
BL = B // NCORES  # 1024
Q = BL // P  # 8
R = 4 * H  # 8 gate rows
# torch gate order [i0 i1 f0 f1 g0 g1 o0 o1] -> ours [f0 f1 i0 i1 o0 o1 g0 g1]
PERM = [2, 3, 0, 1, 6, 7, 4, 5]

_CACHE = {}


def build_nc(q=Q, t_steps=T, i_feat=I, num_devices=NCORES, repeats=1, nchains=2,
             chunk_bytes=2_200_000, pe_accum=False, mode="full", xdt="f16",
             sbufs=2, xinbufs=3, psumbufs=6):
    import concourse.bacc as bacc
    import concourse.mybir as mybir
    import concourse.tile as tile

    f32 = mybir.dt.float32
    f16 = mybir.dt.float16
    fx = mybir.dt.float8e4 if xdt == "f8" else f16
    Alu = mybir.AluOpType
    Act = mybir.ActivationFunctionType

    kf = i_feat + 1  # +1 ones row carries the bias
    ncols = t_steps * q * P
    w_g = q * R  # free width of one timestep's gates
    nc = bacc.Bacc("TRN2", target_bir_lowering=False, debug=False,
                   num_devices=num_devices)

    x2 = nc.dram_tensor("x2", [kf, ncols], fx, kind="ExternalInput")
    wih = nc.dram_tensor("wih", [kf, R], f16, kind="ExternalInput")
    whh = nc.dram_tensor("whh", [P, 2 * w_g], f32, kind="ExternalInput")
    fcw = nc.dram_tensor("fcw", [P, 2 * q], f32, kind="ExternalInput")
    fcb = nc.dram_tensor("fcb", [P, 1], f32, kind="ExternalInput")
    ident = nc.dram_tensor("ident", [P, P], f16, kind="ExternalInput")
    out = nc.dram_tensor("out", [P, q], f32, kind="ExternalOutput")

    per_t_bytes = kf * q * P * (1 if xdt == 'f8' else 2)
    chunk_t = max(1, int(np.ceil(chunk_bytes / per_t_bytes)))
    chunk_t = min(chunk_t, t_steps)
    while t_steps % chunk_t:
        chunk_t -= 1
    tcols = q * P

    with tile.TileContext(nc) as tc:
        with (
            tc.tile_pool(name="consts", bufs=1) as consts,
            tc.tile_pool(name="state", bufs=1) as state,
            tc.tile_pool(name="xin", bufs=xinbufs) as xin,
            tc.tile_pool(name="psum", bufs=psumbufs, space="PSUM") as psum,
            tc.tile_pool(name="tmp", bufs=sbufs) as tmpp,
            tc.tile_pool(name="sg", bufs=sbufs) as sgp,
            tc.tile_pool(name="gg", bufs=sbufs) as ggp,
            tc.tile_pool(name="mm", bufs=sbufs) as mmp,
        ):
            wih_t = consts.tile([kf, R], f16)
            nc.sync.dma_start(wih_t[:], wih[:])
            whh_t = consts.tile([P, 2 * w_g], f32)
            nc.sync.dma_start(whh_t[:], whh[:])
            fcw_t = consts.tile([P, 2 * q], f32)
            nc.sync.dma_start(fcw_t[:], fcw[:])
            fcb_t = consts.tile([P, 1], f32)
            nc.sync.dma_start(fcb_t[:], fcb[:])
            ident_t = consts.tile([P, P], f16)
            nc.sync.dma_start(ident_t[:], ident[:])

            assert q % nchains == 0
            qc = q // nchains  # q-groups per chain
            wc = qc * R        # gate free-width per chain
            hs, Ss = [], []
            for ch in range(nchains):
                hch = state.tile([P, 2 * qc], f32, tag=f"h{ch}")  # (i, q)
                Sch = state.tile([P, 2 * qc], f32, tag=f"S{ch}")  # c, (q, i)
                nc.vector.memset(hch[:], 0.0)
                nc.vector.memset(Sch[:], 0.0)
                hs.append(hch)
                Ss.append(Sch)

            # touch the sigmoid table set immediately so its load overlaps DMA
            warm = state.tile([P, 1], f32, tag="warm")
            nc.scalar.activation(warm[:], hs[0][:, 0:1], Act.Sigmoid)
            ptc = state.tile([P, w_g], f32, tag="ptc")
            nc.vector.memset(ptc[:], 0.1)

            whh_v = whh_t[:].rearrange("p (i q r) -> p i q r", i=2, q=q)

            def scan_pre(ch, s_tiles):
                """Per-chain: produce s_t (h-part of gates) in SBUF.

                s is written fp16 so the PE identity-matmul (fp16 stationary,
                FWL-eligible) can consume it directly."""
                h = hs[ch]
                tmp = tmpp.tile([P, 2 * wc], f32, tag=f"tmp{ch}")
                h_b = h[:].rearrange("p (i q) -> p i q", i=2).broadcast_to(
                    (P, 2, qc, R)
                )
                nc.vector.tensor_tensor(
                    tmp[:].rearrange("p (i q r) -> p i q r", i=2, q=qc),
                    h_b, whh_v[:, :, ch * qc:(ch + 1) * qc, :], Alu.mult,
                )
                s_tiles[ch] = tmp

            def scan_post(ch, g_slice):
                """Per-chain: from complete gates g (PSUM) update c and h."""
                h, S = hs[ch], Ss[ch]
                sg = sgp.tile([P, wc], f32, tag=f"sg{ch}")
                sgv = sg[:].rearrange("p (q r) -> p q r", q=qc)
                nc.scalar.activation(sg[:], g_slice, Act.Sigmoid)
                Sv = S[:].rearrange("p (q i) -> p q i", q=qc)
                # mB = (sig_g - 0.5) * sig_i   [= 0.5 * i_gate * tanh(g)]
                mb = mmp.tile([P, 2 * qc], f32, tag=f"mb{ch}")
                mbv = mb[:].rearrange("p (q i) -> p q i", q=qc)
                nc.vector.scalar_tensor_tensor(
                    mbv, sgv[:, :, 6:8], 0.5, sgv[:, :, 2:4],
                    Alu.subtract, Alu.mult,
                )
                # mA = sig_f * c
                ma = mmp.tile([P, 2 * qc], f32, tag=f"ma{ch}")
                mav = ma[:].rearrange("p (q i) -> p q i", q=qc)
                nc.gpsimd.tensor_tensor(mav, sgv[:, :, 0:2], Sv, Alu.mult)
                # c = 2*mB + mA
                nc.vector.scalar_tensor_tensor(
                    Sv, mbv, 2.0, mav, Alu.mult, Alu.add
                )
                # Tc = tanh(c)
                tcl = mmp.tile([P, 2 * qc], f32, tag=f"tc{ch}")
                tcv = tcl[:].rearrange("p (q i) -> p q i", q=qc)
                nc.scalar.activation(tcv, Sv, Act.Tanh)
                # h = o * Tc   (write into (i,q) layout via transposed view)
                hv = h[:].rearrange("p (i q) -> p q i", i=2)
                nc.gpsimd.tensor_tensor(hv, sgv[:, :, 4:6], tcv, Alu.mult)

            # Front-load a small starter chunk so the first matmuls (and the
            # scan behind them) begin ~4-5us earlier; steady state unchanged.
            if chunk_t >= 4 and t_steps >= 2 * chunk_t:
                schedule = [2, chunk_t - 2] + [chunk_t] * (t_steps // chunk_t - 1)
            else:
                schedule = [chunk_t] * (t_steps // chunk_t)
            assert sum(schedule) == t_steps
            for rep_i in range(repeats):
              t_base = 0
              for csz in schedule:
                xt = xin.tile([kf, csz * tcols], fx, tag=f"xt{csz}")
                nc.sync.dma_start(
                    xt[:], x2[:, t_base * tcols:(t_base + csz) * tcols]
                )
                if mode == "dma":
                    sink = mmp.tile([kf, 1], f32, tag="sink")
                    nc.vector.tensor_copy(sink[:], xt[:, 0:1])
                    nc.vector.tensor_copy(hs[0][0:1, 0:1], sink[0:1, 0:1])
                    t_base += csz
                    continue
                for dt_ in range(csz):
                    t = t_base + dt_
                    s_tiles = [None] * nchains
                    if mode == "scan":
                        for ch in range(nchains):
                            scan_pre(ch, s_tiles)
                        for ch in range(nchains):
                            tmp = s_tiles[ch]
                            s2 = ggp.tile([P, wc], f32, tag=f"s2{ch}")
                            nc.vector.tensor_tensor(
                                s2[:], tmp[:, 0:wc],
                                ptc[:, ch * wc:(ch + 1) * wc], Alu.add)
                            g = ggp.tile([P, wc], f32, tag=f"g{ch}")
                            nc.gpsimd.tensor_tensor(
                                g[:], s2[:], tmp[:, wc:2 * wc], Alu.add)
                            scan_post(ch, g[:])
                        continue
                    if pe_accum:
                        # h-part of gates first: the identity matmul opens the
                        # accumulation group (start=True), the per-q input
                        # projections then accumulate gates_x on top.
                        for ch in range(nchains):
                            scan_pre(ch, s_tiles)
                        pt = psum.tile([P, w_g], f32)
                        for ch in range(nchains):
                            nc.tensor.matmul(
                                pt[:, ch * wc:(ch + 1) * wc],
                                ident_t[:],
                                s_tiles[ch][:],
                                start=True,
                                stop=False,
                            )
                        for qq in range(q):
                            nc.tensor.matmul(
                                pt[:, qq * R:(qq + 1) * R],
                                xt[:, dt_ * tcols + qq * P: dt_ * tcols + (qq + 1) * P],
                                wih_t[:],
                                start=False,
                                stop=True,
                            )
                        for ch in range(nchains):
                            scan_post(ch, pt[:, ch * wc:(ch + 1) * wc])
                    else:
                        pt = psum.tile([P, w_g], f32)
                        for qq in range(q):
                            nc.tensor.matmul(
                                pt[:, qq * R:(qq + 1) * R],
                                xt[:, dt_ * tcols + qq * P: dt_ * tcols + (qq + 1) * P],
                                wih_t[:],
                                start=True,
                                stop=True,
                            )
                        for ch in range(nchains):
                            scan_pre(ch, s_tiles)
                        for ch in range(nchains):
                            tmp = s_tiles[ch]
                            s2 = ggp.tile([P, wc], f32, tag=f"s2{ch}")
                            nc.vector.tensor_tensor(
                                s2[:], tmp[:, 0:wc],
                                pt[:, ch * wc:(ch + 1) * wc], Alu.add)
                            g = ggp.tile([P, wc], f32, tag=f"g{ch}")
                            nc.gpsimd.tensor_tensor(
                                g[:], s2[:], tmp[:, wc:2 * wc], Alu.add)
                            scan_post(ch, g[:])
                t_base += csz

            # --- FC tail: out = h0*w0 + h1*w1 + b ---
            for ch in range(nchains):
                mfc = mmp.tile([P, 2 * qc], f32, tag=f"mfc{ch}")
                nc.vector.tensor_tensor(
                    mfc[:].rearrange("p (i q) -> p i q", i=2),
                    hs[ch][:].rearrange("p (i q) -> p i q", i=2),
                    fcw_t[:].rearrange("p (i q) -> p i q", i=2)[
                        :, :, ch * qc:(ch + 1) * qc],
                    Alu.mult,
                )
                res = mmp.tile([P, qc], f32, tag=f"res{ch}")
                nc.vector.scalar_tensor_tensor(
                    res[:], mfc[:, 0:qc], fcb_t[:, 0:1], mfc[:, qc:2 * qc],
                    Alu.add, Alu.add,
                )
                nc.sync.dma_start(out[:, ch * qc:(ch + 1) * qc], res[:])

    nc.compile()
    return nc


def prep_core_inputs(image_c, W_ih, W_hh, b_ih, b_hh, fc_w, fc_b,
                     q=Q, t_steps=T, i_feat=I, xdt="f16"):
    """Host-side shard prep for one core's batch slice image_c [q*128, T, I]."""
    bl = q * P
    kf = i_feat + 1
    if xdt == "f8":
        import ml_dtypes
        xnp = np.dtype(ml_dtypes.float8_e4m3)
    else:
        xnp = np.dtype(np.float16)
    # x2[(feature), (t, q, b)]
    arr = np.asarray(image_c, xnp).reshape(q, P, t_steps, i_feat)
    arr = arr.transpose(3, 2, 0, 1)
    x2 = np.empty((kf, t_steps * bl), dtype=xnp)
    x2[:i_feat] = arr.reshape(i_feat, t_steps * bl)
    x2[i_feat] = 1.0

    bias = (b_ih + b_hh).astype(np.float32)
    # rows r=6,7 (the cell-candidate gate) are pre-scaled x2 so the kernel can
    # use a single sigmoid table lookup: tanh(x) = 2*sigmoid(2x) - 1.
    rscale = np.array([1.0, 1.0, 1.0, 1.0, 1.0, 1.0, 2.0, 2.0], np.float32)
    wih = np.empty((kf, R), dtype=np.float16)
    for r in range(R):
        wih[:i_feat, r] = (W_ih[PERM[r]] * rscale[r]).astype(np.float16)
        wih[i_feat, r] = np.float16(bias[PERM[r]] * rscale[r])

    whh = np.empty((P, 2 * q * R), dtype=np.float32)
    for i in range(2):
        for qq in range(q):
            for r in range(R):
                whh[:, i * q * R + qq * R + r] = W_hh[PERM[r], i] * rscale[r]

    fcw = np.empty((P, 2 * q), dtype=np.float32)
    for i in range(2):
        fcw[:, i * q:(i + 1) * q] = fc_w[0, i]
    fcb = np.full((P, 1), fc_b[0], dtype=np.float32)
    ident = np.eye(P, dtype=np.float16)
    return {"x2": np.ascontiguousarray(x2), "wih": wih, "whh": whh,
            "fcw": fcw, "fcb": fcb, "ident": ident}


def kernel(image, W_ih, W_hh, b_ih, b_hh, fc_w, fc_b):
    from concourse.bass_utils import run_bass_kernel_spmd

    image = np.asarray(image, dtype=np.float32)
    if "nc" not in _CACHE:
        _CACHE["nc"] = build_nc(nchains=1, sbufs=6, xinbufs=4, psumbufs=8)
    nc = _CACHE["nc"]

    in_maps = []
    for c in range(NCORES):
        in_maps.append(
            prep_core_inputs(
                image[c * BL:(c + 1) * BL],
                np.asarray(W_ih, np.float32), np.asarray(W_hh, np.float32),
                np.asarray(b_ih, np.float32), np.asarray(b_hh, np.float32),
                np.asarray(fc_w, np.float32), np.asarray(fc_b, np.float32),
            )
        )
    res = run_bass_kernel_spmd(nc, in_maps, list(range(NCORES)))
    outp = np.empty((B, 1), dtype=np.float32)
    for c in range(NCORES):
        oc = res.results[c]["out"]  # [128, Q] -> b_local = q*128 + p
        outp[c * BL:(c + 1) * BL, 0] = oc.T.reshape(BL)
    return outp
